# revision 11
# baseline (speedup 1.0000x reference)
"""Bass/Trainium2 kernel for nn_BiLSTMWindowModel.

Model: xv = [tanh(x), sigmoid(x)]; 2-layer BiLSTM (H=256); head on last timestep.

Strategy (8 NeuronCores, data-parallel over batch, 64 rows/core):
  - Gate order permuted to [g, i, o, f] so tanh / sigmoid-bf16 / sigmoid-f32
    each hit one contiguous block.
  - Batch-major gates in PSUM [128, 1024]: layer-0 fwd chain in partitions
    0:64, bwd chain in partitions 64:128 (concurrent col-tiled matmuls).
  - Matmuls in bf16 (1 cycle/row); cell state c kept in fp32; the f-gate is
    consumed in fp32 so the c recurrence only sees bf16 rounding through the
    i/g/o gates and h.
  - Input projections computed directly into the gates PSUM tile
    (start=True), recurrent matmuls accumulate on top (start=False).
    Biases folded in via an appended ones-row on the contraction dim.
  - h kept transposed (feature-major) via PE transpose for the next
    step's stationary operand; h0^T streamed to DRAM for layer 1.
  - Layer-1 backward direction needs only its first step (t=T-1), since
    the head reads h1[:, -1, :].
"""

import numpy as np
import ml_dtypes
from contextlib import ExitStack

import concourse.bacc as bacc
import concourse.bass as bass
import concourse.mybir as mybir
import concourse.tile as tile
from concourse.bass_utils import run_bass_kernel_spmd

F32 = mybir.dt.float32
BF16 = mybir.dt.bfloat16
AFT = mybir.ActivationFunctionType
NPBF = ml_dtypes.bfloat16

B, T, D_IN, H, D_OUT = 512, 200, 40, 256, 2
NCORES = 8
BC = B // NCORES          # 64 batch rows per core
G4 = 4 * H                # 1024

# gate permutation: torch order [i,f,g,o] -> ours [g,i,o,f]
PERM = np.concatenate([np.arange(2 * H, 3 * H), np.arange(0, H),
                       np.arange(3 * H, 4 * H), np.arange(H, 2 * H)])
SG = slice(0, H)          # tanh gate g (gates cols 0:256)
# io block: gates cols 256:768 (i=0:256, o=256:512 of io_sb); f: cols 768:1024
# xv layout (SBUF partition bases must be 32-aligned):
#   rows 0:40 tanh(x), 40:64 zeros, 64:104 sigmoid(x), 104 ones (bias row)
XROWS = 105


def build(t_steps=T):
    """Build + compile the Bass program for all 8 cores (SPMD)."""
    tt = t_steps
    cols = tt * BC
    nc = bacc.Bacc("TRN2", target_bir_lowering=False, debug=False,
                   num_devices=NCORES)

    # ---- DRAM I/O ----
    d_xT = nc.dram_tensor("xT", [D_IN, cols], F32, kind="ExternalInput")
    d_ih0 = {s: nc.dram_tensor(f"ih0_{s}", [XROWS, G4], BF16,
                               kind="ExternalInput") for s in ("f", "b")}
    d_hh0 = {s: nc.dram_tensor(f"hh0_{s}", [128, 2 * G4], BF16,
                               kind="ExternalInput") for s in ("f", "b")}
    d_ih1m = {s: nc.dram_tensor(f"ih1m_{s}", [128, 4 * G4], BF16,
                                kind="ExternalInput") for s in ("f", "b")}
    d_ih1b = {s: nc.dram_tensor(f"ih1b_{s}", [1, G4], BF16,
                                kind="ExternalInput") for s in ("f", "b")}
    d_hh1 = nc.dram_tensor("hh1_f", [128, 2 * G4], BF16, kind="ExternalInput")
    d_headm = nc.dram_tensor("head_m", [128, 4 * D_OUT], BF16,
                             kind="ExternalInput")
    d_headb = nc.dram_tensor("head_b", [1, D_OUT], BF16, kind="ExternalInput")
    d_id2 = nc.dram_tensor("id2", [128, 64], BF16, kind="ExternalInput")
    d_out = nc.dram_tensor("outT", [D_OUT, BC], F32, kind="ExternalOutput")

    with tile.TileContext(nc) as tc, ExitStack() as ctx:
        wp = ctx.enter_context(tc.tile_pool(name="wts", bufs=1))
        gp = ctx.enter_context(tc.tile_pool(name="gates", bufs=2, space="PSUM"))
        tp = ctx.enter_context(tc.tile_pool(name="tps", bufs=2, space="PSUM"))
        sp = ctx.enter_context(tc.tile_pool(name="work", bufs=3))
        xp = ctx.enter_context(tc.tile_pool(name="xin", bufs=2))
        hp = ctx.enter_context(tc.tile_pool(name="hblk", bufs=2))
        dp = ctx.enter_context(tc.tile_pool(name="dram", bufs=1, space="DRAM"))

        # h0 transposed, feature-major: 4 K-tiles of [128, cols] in DRAM
        h0T = [dp.tile([128, cols], BF16, tag=f"h0T{k}", name=f"h0T{k}")
               for k in range(4)]

        # ---- load weights/constants into SBUF ----
        def wtile(dram, shape, tag):
            t_ = wp.tile(shape, BF16, tag=tag, name=tag)
            nc.sync.dma_start(t_[:, :], dram.ap())
            return t_

        ih0 = {s: wtile(d_ih0[s], [XROWS, G4], f"ih0{s}") for s in ("f", "b")}
        hh0 = {s: wtile(d_hh0[s], [128, 2 * G4], f"hh0{s}") for s in ("f", "b")}
        ih1m = {s: wtile(d_ih1m[s], [128, 4 * G4], f"ih1m{s}")
                for s in ("f", "b")}
        ih1b = {s: wtile(d_ih1b[s], [1, G4], f"ih1b{s}") for s in ("f", "b")}
        hh1 = wtile(d_hh1, [128, 2 * G4], "hh1")
        headm = wtile(d_headm, [128, 4 * D_OUT], "headm")
        headb = wtile(d_headb, [1, D_OUT], "headb")
        id2 = wtile(d_id2, [128, 64], "id2")
        ones = wp.tile([1, BC], BF16, tag="ones")
        nc.vector.memset(ones[:, :], 1.0)

        # ---- feature view xv rows: 0:40 tanh, 40:64 zero, 64:104 sig,
        # 104 ones/bias row (105:128 unused but memset) ----
        xv = wp.tile([128, cols], BF16, tag="xv")
        nc.vector.memset(xv[32:64, :], 0.0)
        nc.vector.memset(xv[96:128, :], 1.0)
        nblk = max(1, min(8, tt))
        bs = (cols // nblk + BC - 1) // BC * BC
        pos = 0
        while pos < cols:
            w = min(bs, cols - pos)
            xb = xp.tile([D_IN, bs], F32, tag="xb")
            nc.sync.dma_start(xb[:, 0:w], d_xT.ap()[:, pos:pos + w])
            nc.scalar.activation(xv[0:D_IN, pos:pos + w], xb[:, 0:w], AFT.Tanh)
            nc.scalar.activation(xv[64:64 + D_IN, pos:pos + w], xb[:, 0:w],
                                 AFT.Sigmoid)
            pos += w

        def whh_kt(w, k):
            return w[:, k * G4:(k + 1) * G4]

        def cell_tail(g, rows, c_prev, tagsuf=""):
            """Gate nonlinearities + c/h update. Returns (c_tile, h_sb)."""
            g_sb = sp.tile([128, H], BF16, tag="g_sb")
            nc.scalar.activation(g_sb[rows, :], g[rows, SG], AFT.Tanh)
            io_sb = sp.tile([128, 2 * H], BF16, tag="io_sb")
            nc.scalar.activation(io_sb[rows, :], g[rows, H:3 * H], AFT.Sigmoid)
            f_sb = sp.tile([128, H], F32, tag="f_sb")
            nc.scalar.activation(f_sb[rows, :], g[rows, 3 * H:G4], AFT.Sigmoid)
            t2 = sp.tile([128, H], F32, tag="t2")
            nc.vector.tensor_mul(t2[rows, :], io_sb[rows, 0:H], g_sb[rows, :])
            c = sp.tile([128, H], F32, tag="c")
            if c_prev is None:
                nc.vector.tensor_copy(c[rows, :], t2[rows, :])
            else:
                t1 = sp.tile([128, H], F32, tag="t1")
                nc.vector.tensor_mul(t1[rows, :], f_sb[rows, :],
                                     c_prev[rows, :])
                nc.vector.tensor_add(c[rows, :], t1[rows, :], t2[rows, :])
            tc_sb = sp.tile([128, H], BF16, tag="tc_sb")
            nc.scalar.activation(tc_sb[rows, :], c[rows, :], AFT.Tanh)
            h_sb = sp.tile([128, H], BF16, tag="h_sb")
            nc.vector.tensor_mul(h_sb[rows, :], io_sb[rows, H:2 * H],
                                 tc_sb[rows, :])
            return c, h_sb

        # ================= Phase 1: layer 0, fwd(0:64) + bwd(64:128) ========
        cprev = None
        hTf = hTb = None
        for s in range(tt):
            t_f, t_b = s, tt - 1 - s
            g = gp.tile([128, G4], F32, tag="g")
            for n in range(2):
                nsl = slice(n * 512, (n + 1) * 512)
                nc.tensor.matmul(g[0:64, nsl],
                                 xv[0:XROWS, t_f * BC:(t_f + 1) * BC],
                                 ih0["f"][:, nsl], start=True, stop=(s == 0))
                nc.tensor.matmul(g[64:128, nsl],
                                 xv[0:XROWS, t_b * BC:(t_b + 1) * BC],
                                 ih0["b"][:, nsl], start=True, stop=(s == 0))
                if s > 0:
                    for k in range(2):
                        nc.tensor.matmul(g[0:64, nsl],
                                         hTf[:, k * 64:(k + 1) * 64],
                                         whh_kt(hh0["f"], k)[:, nsl],
                                         start=False, stop=(k == 1))
                        nc.tensor.matmul(g[64:128, nsl],
                                         hTb[:, k * 64:(k + 1) * 64],
                                         whh_kt(hh0["b"], k)[:, nsl],
                                         start=False, stop=(k == 1))

            cprev, h_sb = cell_tail(g, slice(0, 128), cprev)

            ht_f = tp.tile([128, 128], BF16, tag="htf")
            ht_b = tp.tile([128, 128], BF16, tag="htb")
            for k in range(2):
                nc.tensor.transpose(ht_f[:, k * 64:(k + 1) * 64],
                                    h_sb[0:64, k * 128:(k + 1) * 128],
                                    id2[0:64, :])
                nc.tensor.transpose(ht_b[:, k * 64:(k + 1) * 64],
                                    h_sb[64:128, k * 128:(k + 1) * 128],
                                    id2[64:128, :])
            hTf = sp.tile([128, 128], BF16, tag="hTf")
            nc.vector.tensor_copy(hTf[:, :], ht_f[:, :])
            hTb = sp.tile([128, 128], BF16, tag="hTb")
            nc.scalar.copy(hTb[:, :], ht_b[:, :])
            for k in range(2):
                nc.sync.dma_start(h0T[k][:, t_f * BC:(t_f + 1) * BC],
                                  hTf[:, k * 64:(k + 1) * 64])
                nc.sync.dma_start(h0T[2 + k][:, t_b * BC:(t_b + 1) * BC],
                                  hTb[:, k * 64:(k + 1) * 64])

        # ================= Phase 2a: layer 1 bwd, single step t=tt-1 ========
        rows = slice(0, 64)
        h199 = wp.tile([128, 4 * BC], BF16, tag="h199")
        for k in range(4):
            nc.sync.dma_start(h199[:, k * BC:(k + 1) * BC],
                              h0T[k][:, (tt - 1) * BC:tt * BC])
        gb = gp.tile([128, G4], F32, tag="g")
        for n in range(2):
            nsl = slice(n * 512, (n + 1) * 512)
            for k in range(4):
                nc.tensor.matmul(gb[rows, nsl],
                                 h199[:, k * BC:(k + 1) * BC],
                                 ih1m["b"][:, k * G4 + n * 512:
                                           k * G4 + (n + 1) * 512],
                                 start=(k == 0), stop=False)
            nc.tensor.matmul(gb[rows, nsl], ones[:, :], ih1b["b"][:, nsl],
                             start=False, stop=True)
        _, h1b = cell_tail(gb, rows, None)
        h1bT_ps = tp.tile([128, 128], BF16, tag="htb")
        for k in range(2):
            nc.tensor.transpose(h1bT_ps[:, k * 64:(k + 1) * 64],
                                h1b[0:64, k * 128:(k + 1) * 128], id2[0:64, :])
        h1bT = wp.tile([128, 128], BF16, tag="h1bT")
        nc.scalar.copy(h1bT[:, :], h1bT_ps[:, :])

        # ================= Phase 2b: layer 1 fwd chain ======================
        BLK = 8 if tt >= 8 else tt
        c1 = None
        hT1 = None
        for blk in range((tt + BLK - 1) // BLK):
            s0 = blk * BLK
            ns = min(BLK, tt - s0)
            hb = []
            for k in range(4):
                hbk = hp.tile([128, BLK * BC], BF16, tag=f"hb{k}",
                              name=f"hb{k}")
                nc.sync.dma_start(hbk[:, 0:ns * BC],
                                  h0T[k][:, s0 * BC:(s0 + ns) * BC])
                hb.append(hbk)
            for ss in range(ns):
                s = s0 + ss
                g = gp.tile([128, G4], F32, tag="g")
                for n in range(2):
                    nsl = slice(n * 512, (n + 1) * 512)
                    for k in range(4):
                        nc.tensor.matmul(
                            g[rows, nsl],
                            hb[k][:, ss * BC:(ss + 1) * BC],
                            ih1m["f"][:, k * G4 + n * 512:
                                      k * G4 + (n + 1) * 512],
                            start=(k == 0), stop=False)
                    nc.tensor.matmul(g[rows, nsl], ones[:, :],
                                     ih1b["f"][:, nsl],
                                     start=False, stop=(s == 0))
                    if s > 0:
                        for k in range(2):
                            nc.tensor.matmul(g[rows, nsl],
                                             hT1[:, k * 64:(k + 1) * 64],
                                             whh_kt(hh1, k)[:, nsl],
                                             start=False, stop=(k == 1))
                c1, h_sb = cell_tail(g, rows, c1)
                ht_f = tp.tile([128, 128], BF16, tag="htf")
                for k in range(2):
                    nc.tensor.transpose(ht_f[:, k * 64:(k + 1) * 64],
                                        h_sb[0:64, k * 128:(k + 1) * 128],
                                        id2[0:64, :])
                hT1 = sp.tile([128, 128], BF16, tag="hTf")
                nc.vector.tensor_copy(hT1[:, :], ht_f[:, :])

        # ================= Phase 3: head ====================================
        op = tp.tile([D_OUT, BC], F32, tag="htf")
        srcs = [hT1[:, 0:64], hT1[:, 64:128], h1bT[:, 0:64], h1bT[:, 64:128]]
        for k in range(4):
            nc.tensor.matmul(op[:, :], headm[:, k * D_OUT:(k + 1) * D_OUT],
                             srcs[k], start=(k == 0), stop=False)
        nc.tensor.matmul(op[:, :], headb[:, :], ones[:, :],
                         start=False, stop=True)
        out_sb = sp.tile([D_OUT, BC], F32, tag="out_sb")
        nc.vector.tensor_copy(out_sb[:, :], op[:, :])
        nc.sync.dma_start(d_out.ap(), out_sb[:, :])

    nc.compile()
    return nc


# =============================== host side ===============================

def _bf(a):
    return np.ascontiguousarray(np.asarray(a, np.float32).astype(NPBF))


def prep_weights(inp):
    """Host-side layout prep (pure layout/marshalling, replicated per core)."""
    def paug(w_ih, w_hh, b_ih, b_hh, l0=False):
        wi = np.asarray(w_ih)[PERM]          # [1024, din]
        wh = np.asarray(w_hh)[PERM]          # [1024, 256]
        bb = (np.asarray(b_ih) + np.asarray(b_hh))[PERM]  # [1024]
        if l0:
            rhs_ih = np.zeros((XROWS, G4), np.float32)
            rhs_ih[0:D_IN] = wi.T[0:D_IN]
            rhs_ih[64:64 + D_IN] = wi.T[D_IN:2 * D_IN]
            rhs_ih[XROWS - 1] = bb
        else:
            rhs_ih = np.concatenate([wi.T, bb[None, :]], axis=0)
        whhT = np.ascontiguousarray(wh.T)    # [256, 1024]
        whh_t = np.ascontiguousarray(
            whhT.reshape(2, 128, G4).transpose(1, 0, 2).reshape(128, 2 * G4))
        return _bf(rhs_ih), _bf(whh_t)

    out = {}
    out["ih0_f"], out["hh0_f"] = paug(inp["w_ih_l0"], inp["w_hh_l0"],
                                      inp["b_ih_l0"], inp["b_hh_l0"], l0=True)
    out["ih0_b"], out["hh0_b"] = paug(inp["w_ih_l0_r"], inp["w_hh_l0_r"],
                                      inp["b_ih_l0_r"], inp["b_hh_l0_r"],
                                      l0=True)
    ih1f, out["hh1_f"] = paug(inp["w_ih_l1"], inp["w_hh_l1"],
                              inp["b_ih_l1"], inp["b_hh_l1"])
    ih1b, _ = paug(inp["w_ih_l1_r"], inp["w_hh_l1_r"],
                   inp["b_ih_l1_r"], inp["b_hh_l1_r"])
    for s, m in (("f", np.asarray(ih1f, np.float32)),
                 ("b", np.asarray(ih1b, np.float32))):
        main = m[:512]                        # [512, 1024]
        out[f"ih1m_{s}"] = _bf(
            main.reshape(4, 128, G4).transpose(1, 0, 2).reshape(128, 4 * G4))
        out[f"ih1b_{s}"] = _bf(m[512:513])
    hm = np.asarray(inp["head_w"]).T          # [512, 2]
    out["head_m"] = _bf(
        hm.reshape(4, 128, D_OUT).transpose(1, 0, 2).reshape(128, 4 * D_OUT))
    out["head_b"] = _bf(np.asarray(inp["head_b"])[None, :])
    out["id2"] = _bf(np.tile(np.eye(64, dtype=np.float32), (2, 1)))
    return out


_BUILD_CACHE = {}


def _get_nc(tt):
    if tt not in _BUILD_CACHE:
        _BUILD_CACHE[tt] = build(tt)
    return _BUILD_CACHE[tt]


def run(inputs, t_steps=T, trace=False):
    """inputs: full-size dict as from setup_inputs (x may have T=t_steps)."""
    x = np.asarray(inputs["x"], dtype=np.float32)
    assert x.shape == (B, t_steps, D_IN), x.shape
    wmaps = prep_weights(inputs)
    nc = _get_nc(t_steps)
    in_maps = []
    for c in range(NCORES):
        xs = x[c * BC:(c + 1) * BC]                      # [64, tt, 40]
        xT = np.ascontiguousarray(
            xs.transpose(2, 1, 0).reshape(D_IN, t_steps * BC), dtype=np.float32)
        m = dict(wmaps)
        m["xT"] = xT
        in_maps.append(m)
    res = run_bass_kernel_spmd(nc, in_maps, core_ids=list(range(NCORES)),
                               trace=trace)
    outs = [res.results[c]["outT"].T for c in range(NCORES)]  # [64, 2] each
    full = np.concatenate(outs, axis=0).astype(np.float32)
    return full, res


def kernel(**inputs):
    out, _ = run(inputs, t_steps=T)
    return out
